# revision 50
# baseline (speedup 1.0000x reference)
"""BrainGCN Trainium2 kernel v2: 2x GCNConv + 3 FC layers over a 100K-node,
1.6M-edge random graph, distributed over 8 NeuronCores.

Strategy (v2, transform-first + binary one-hot):
- Nodes (dst) sharded across 8 cores; per core 12544 slots in 98 blocks of 128,
  assigned by capped vector-LPT (per-bucket in-edge counts balanced, hard cap
  512 per (bucket, block) so most groups need exactly 4 tiles).
- Transform-first: each layer gathers from a table T_l[v] = dinv[v]*(h@W)[v]
  (row-major, 64 real bf16 feats in 256B rows; upper half garbage, never read).
  Tables are device-built (PE matmul per block + scalar-engine scale) and
  exchanged with AllGather. dinv[src] is folded into the table, dinv[dst] is a
  per-partition activation scale, the self-loop is "one more edge" (identity
  matmul over the core's own shard rows), so the per-edge one-hot S is BINARY:
  one cheap DVE is_equal per tile, no per-edge weights anywhere.
- Aggregation runs slot-major: psum[slot, feat] += S_t.T @ G_t for all tiles of
  a block in ONE psum run (no SBUF accumulators). Blocks processed in groups of
  6 so gather calls chunk long same-bucket segments.
- Buckets are 4x25088 rows (= 2 shards) so an edge's bucket is src_core//2 —
  known before LPT, and int16 gather indices stay in range.
- Messages fetched with gpsimd dma_gather (256B/edge), CH tiles per call,
  single_packet=False, round-robined over 4 SWDGE queues.

Host-side work is limited to graph-structure preprocessing (degrees, norms,
permutations, index/metadata arrays, row reordering/casting of x) and final
unpermutation.
"""

import os
import sys
import types

import numpy as np


def _install_ntff_hook():
    """Image's antenv lacks axon_hooks; shim it so trace=True can profile."""
    if "antenv.axon_hooks" in sys.modules:
        return
    mod = types.ModuleType("antenv.axon_hooks")
    mod._hook = None
    mod.set_axon_ntff_profile_hook = lambda h: setattr(mod, "_hook", h)
    mod.get_axon_ntff_profile_hook = lambda: mod._hook
    sys.modules["antenv.axon_hooks"] = mod
    try:
        import antenv
        antenv.axon_hooks = mod
        from trn_agent_boot.trn_boot import _ntff_profile_via_ctypes
        mod.set_axon_ntff_profile_hook(
            _ntff_profile_via_ctypes("/opt/axon/libaxon_pjrt.so")
        )
    except Exception:
        pass


_install_ntff_hook()

import ml_dtypes
import concourse.bacc as bacc
import concourse.bass as bass  # noqa: F401
import concourse.mybir as mybir
import concourse.tile as tile
from concourse.bass_utils import run_bass_kernel_spmd

# ---------------------------------------------------------------- constants
N = 100000
D_IN = 128
H1 = 64
NCORES = 8
SHARD = N // NCORES            # 12500
BLKN = 98                      # blocks of 128 slots per core
SLOTS = BLKN * 128             # 12544
NROWS = NCORES * SLOTS         # 100352 table rows
NBUCK = 4
BSIZE = 2 * SLOTS              # 25088 rows per bucket (= 2 src shards)
CAP = 512                      # LPT hard cap per (bucket, block): 4 tiles
G = int(os.environ.get("BASS_GCN_G", "6"))      # psum rotation depth
SG = int(os.environ.get("BASS_GCN_SG", "3"))     # blocks per schedule group
CH = int(os.environ.get("BASS_GCN_CH", "16"))   # tiles per dma_gather call
GATB = int(os.environ.get("BASS_GCN_GATB", "18"))  # gather tile bufs

LAST_EXEC_TIME_NS = None       # filled when BASS_GCN_TRACE=1
LAST_RESULTS = None


# ------------------------------------------------------------- host planning
def _lpt_assign_vec(loads, capsT):
    """Pack nodes into BLKN blocks x 128 slots, balancing the per-bucket load
    vectors (sum-of-squares greedy, descending total load) under per-(bucket,
    block) caps (a few designated overflow blocks get a 640 cap)."""
    n = loads.shape[0]
    order = np.argsort(-loads.sum(1), kind="stable")
    block_loads = np.zeros((BLKN, loads.shape[1]), np.float64)
    used = np.zeros(BLKN, np.int64)
    pos = np.empty(n, np.int64)
    for i in order:
        li = loads[i]
        cand = block_loads + li
        score = np.einsum("ij,ij->i", cand, cand)
        over = (cand > capsT).sum(axis=1)
        score = score + over * 1e12
        score[used >= 128] = np.inf
        b = int(np.argmin(score))
        pos[i] = b * 128 + used[b]
        block_loads[b] += li
        used[b] += 1

    # swap-repair: drive every (block, bucket) under its cap
    blk_of = pos // 128
    stuck = np.zeros((BLKN, loads.shape[1]), bool)
    for _ in range(8000):
        over = np.where(stuck, -1.0, block_loads - capsT)
        b = int(np.argmax(over.max(1)))
        be = int(np.argmax(over[b]))
        if over[b, be] <= 0:
            break
        in_b = np.nonzero(blk_of == b)[0]
        cand_i = in_b[np.argsort(-loads[in_b, be])[:8]]
        done = False
        for b2 in np.argsort(block_loads[:, be]):
            if b2 == b:
                continue
            in_b2 = np.nonzero(blk_of == b2)[0]
            cand_j = in_b2[np.argsort(loads[in_b2, be])[:8]]
            for i in cand_i:
                for j in cand_j:
                    dlt = loads[i] - loads[j]
                    if dlt[be] <= 0:
                        continue
                    nb2 = block_loads[b2] + dlt
                    nb = block_loads[b] - dlt
                    if (nb2 <= capsT[b2]).all() and nb[be] < block_loads[b, be]:
                        block_loads[b2] = nb2
                        block_loads[b] = nb
                        blk_of[i], blk_of[j] = b2, b
                        done = True
                        break
                if done:
                    break
            if done:
                break
        if not done:
            stuck[b, be] = True
    # reassign slots from (possibly updated) block membership
    used = np.zeros(BLKN, np.int64)
    for i in range(n):
        b = blk_of[i]
        pos[i] = b * 128 + used[b]
        used[b] += 1
    return pos


def _plan(src, dst, x):
    """Host-side graph preprocessing. Returns per-core stream data plus the
    shared tile schedule."""
    deg = (np.bincount(dst, minlength=N) + 1.0).astype(np.float64)
    dinv = (1.0 / np.sqrt(deg)).astype(np.float32)

    src_core = src // SHARD
    be_e = src_core // 2                       # bucket of every edge (0..3)

    loads = np.zeros((N, NBUCK), np.float64)
    np.add.at(loads, (dst, be_e), 1.0)

    # overflow blocks: enough 640-cap blocks per bucket to absorb every
    # core's excess over 98x512, +1 margin; disjoint across buckets
    totals = np.zeros((NCORES, NBUCK))
    for c in range(NCORES):
        totals[c] = loads[c * SHARD:(c + 1) * SHARD].sum(0)
    kov = np.maximum(
        0, np.ceil((totals.max(0) - BLKN * CAP) / 128)
    ).astype(int) + 2
    caps = np.full((NBUCK, BLKN), float(CAP))
    m0 = 0
    for be in range(NBUCK):
        caps[be, m0:m0 + kov[be]] = CAP + 128.0
        m0 += kov[be]
    capsT = np.ascontiguousarray(caps.T)

    pos_local = np.empty(N, np.int64)
    node_of_pos = np.full((NCORES, SLOTS), -1, np.int64)
    for c in range(NCORES):
        nodes = np.arange(c * SHARD, (c + 1) * SHARD)
        p = _lpt_assign_vec(loads[nodes], capsT)
        pos_local[nodes] = p
        node_of_pos[c, p] = nodes
    pos_global = (np.arange(N) // SHARD) * SLOTS + pos_local

    rows = pos_global[src]                     # table row per edge
    assert (rows // BSIZE == be_e).all()
    idx_local = rows - be_e * BSIZE            # 0..25087, int16-safe

    core_of = dst // SHARD
    counts = np.zeros((NCORES, NBUCK, BLKN), np.int64)
    for c in range(NCORES):
        m = core_of == c
        blk = pos_local[dst[m]] // 128
        counts[c] = np.bincount(
            be_e[m] * BLKN + blk, minlength=NBUCK * BLKN
        ).reshape(NBUCK, BLKN)

    T = np.ceil(counts.max(axis=0) / 128).astype(np.int64)   # [NBUCK, BLKN]

    # ---- schedule: groups of G blocks, bucket-sweep inside each group
    sched = []                                  # (be, b) emission order
    for g0 in range(0, BLKN, SG):
        blocks = range(g0, min(g0 + SG, BLKN))
        for be in range(NBUCK):
            for b in blocks:
                sched.append((be, b))
    tile_off = np.zeros((NBUCK, BLKN), np.int64)
    off = 0
    for (be, b) in sched:
        tile_off[be, b] = off
        off += T[be, b]
    ntiles = int(off)
    P = 128 * ntiles

    tile_to_block = np.zeros(ntiles, np.int64)
    for (be, b) in sched:
        tile_to_block[tile_off[be, b]: tile_off[be, b] + T[be, b]] = b
    last_tile_of_block = np.full(BLKN, -1, np.int64)
    for (be, b) in sched:
        if T[be, b] > 0:
            last_tile_of_block[b] = tile_off[be, b] + T[be, b] - 1

    # ---- gather chunk lists per group: (tile_start, k, be, comb_off)
    chunks_by_group = []
    comb_off = 0
    for g0 in range(0, BLKN, SG):
        blocks = list(range(g0, min(g0 + SG, BLKN)))
        gch = []
        for be in range(NBUCK):
            seg_len = int(sum(T[be, b] for b in blocks))
            if seg_len == 0:
                continue
            seg_start = int(tile_off[be, blocks[0]])
            t = 0
            while t < seg_len:
                k = min(CH, seg_len - t)
                gch.append((seg_start + t, k, be, comb_off))
                comb_off += 9 * k
                t += k
        chunks_by_group.append((blocks, gch))
    comb_len = comb_off

    # ---- per-core streams
    group_idx = np.zeros((NBUCK, BLKN), np.int64)
    for i, (be, b) in enumerate(sched):
        group_idx[be, b] = i
    dest_base = np.zeros(len(sched) + 1, np.int64)
    np.cumsum([128 * T[be, b] for (be, b) in sched], out=dest_base[1:])

    streams = []
    xpTs = []
    dinvbs = []
    for c in range(NCORES):
        m = core_of == c
        e_rows = idx_local[m]
        e_be = be_e[m]
        e_slot = pos_local[dst[m]]
        key = group_idx[e_be, e_slot // 128]
        order = np.lexsort((e_rows, key))

        skey = key[order]
        cnt = np.bincount(skey, minlength=len(sched))
        starts = np.zeros(len(sched) + 1, np.int64)
        np.cumsum(cnt, out=starts[1:])
        rank = np.arange(len(order)) - starts[skey]
        dest = dest_base[skey] + rank

        out_idx = np.zeros(P, np.int64)
        out_slot = np.full(P, -1.0, np.float32)
        out_idx[dest] = e_rows[order]
        out_slot[dest] = (e_slot[order] % 128).astype(np.float32)

        idx_wrapped = np.tile(
            out_idx.astype(np.int16).reshape(-1, 16).T, (8, 1)
        )                                           # [128, P//16]
        meta = out_slot.reshape(ntiles, 128).T.astype(
            ml_dtypes.bfloat16
        ).view(np.int16)                            # [128, ntiles]
        # combined per-chunk stream: [idx (k*8 cols) | meta (k cols)] int16
        comb = np.zeros((128, comb_len), np.int16)
        for (t0, k, be, off) in [c for (_, gch) in chunks_by_group
                                 for c in gch]:
            comb[:, off: off + 8 * k] = idx_wrapped[:, t0 * 8: (t0 + k) * 8]
            comb[:, off + 8 * k: off + 9 * k] = meta[:, t0: t0 + k]
        streams.append(np.ascontiguousarray(comb))

        xp = np.zeros((SLOTS, D_IN), np.float32)
        valid = node_of_pos[c] >= 0
        xp[valid] = x[node_of_pos[c][valid]]
        xpTs.append(np.ascontiguousarray(xp.T.astype(ml_dtypes.bfloat16)))

        dv = np.zeros(SLOTS, np.float32)
        dv[valid] = dinv[node_of_pos[c][valid]]
        dinvbs.append(np.ascontiguousarray(dv.reshape(BLKN, 128).T))

    plan = {
        "T": T, "ntiles": ntiles, "P": P, "comb_len": comb_len,
        "chunks_by_group": chunks_by_group,
        "tile_to_block": tile_to_block,
        "last_tile_of_block": last_tile_of_block,
        "node_of_pos": node_of_pos,
    }
    return plan, streams, xpTs, dinvbs


# ------------------------------------------------------------ device program
def _build_program(plan):
    f32 = mybir.dt.float32
    gdt = mybir.dt.bfloat16
    scratch = int(os.environ.get("BASS_GCN_SCRATCH", "16384"))
    nc = bacc.Bacc(
        "TRN2", num_swdge_queues=4, dynamic_dma_scratch_size=scratch
    )

    ntiles = plan["ntiles"]
    P = plan["P"]
    chunks_by_group = plan["chunks_by_group"]
    tile_to_block = plan["tile_to_block"]
    last_tile = plan["last_tile_of_block"]

    comb_len = plan["comb_len"]
    xpT_d = nc.dram_tensor("xpT", [D_IN, SLOTS], gdt, kind="ExternalInput")
    comb_d = nc.dram_tensor(
        "comb", [128, comb_len], mybir.dt.int16, kind="ExternalInput"
    )
    dinv_d = nc.dram_tensor("dinvb", [128, BLKN], f32, kind="ExternalInput")
    iota_d = nc.dram_tensor("iota", [128, CH * 128], gdt, kind="ExternalInput")
    identb_d = nc.dram_tensor("identb", [128, 128], gdt, kind="ExternalInput")
    identf_d = nc.dram_tensor("identf", [128, 128], f32, kind="ExternalInput")
    cW0_d = nc.dram_tensor("cW0b", [D_IN, H1], gdt, kind="ExternalInput")
    cW1_d = nc.dram_tensor("cW1b", [H1, H1], gdt, kind="ExternalInput")
    fW0_d = nc.dram_tensor("fW0", [H1, H1], f32, kind="ExternalInput")
    fb0_d = nc.dram_tensor("fb0", [H1, 1], f32, kind="ExternalInput")
    fW1_d = nc.dram_tensor("fW1", [H1, 32], f32, kind="ExternalInput")
    fb1_d = nc.dram_tensor("fb1", [32, 1], f32, kind="ExternalInput")
    fW2_d = nc.dram_tensor("fW2", [32, 1], f32, kind="ExternalInput")
    fb2_d = nc.dram_tensor("fb2", [1, 1], f32, kind="ExternalInput")
    y_d = nc.dram_tensor("y", [BLKN, 128], f32, kind="ExternalOutput")

    with tile.TileContext(nc) as tc:
        with (
            tc.tile_pool(name="cst", bufs=1) as cst,
            tc.tile_pool(name="gatp", bufs=GATB) as gatp,
            tc.tile_pool(name="sp", bufs=10) as sp,
            tc.tile_pool(name="iop", bufs=24) as iop,
            tc.tile_pool(name="hp", bufs=8) as hp,
            tc.tile_pool(name="twp", bufs=1) as twp,
            tc.tile_pool(name="ps_run", bufs=1, space="PSUM") as ps_run,
            tc.tile_pool(name="ps_epi", bufs=2, space="PSUM") as ps_epi,
            tc.tile_pool(name="dram", bufs=1, space="DRAM") as dram,
        ):
            iota_t = cst.tile([128, CH * 128], gdt)
            nc.sync.dma_start(iota_t[:], iota_d[:])
            identb_t = cst.tile([128, 128], gdt)
            nc.sync.dma_start(identb_t[:], identb_d[:])
            identf_t = cst.tile([128, 128], f32)
            nc.sync.dma_start(identf_t[:], identf_d[:])
            dinv_t = cst.tile([128, BLKN], f32)
            nc.sync.dma_start(dinv_t[:], dinv_d[:])
            cW0_t = cst.tile([D_IN, H1], gdt)
            nc.sync.dma_start(cW0_t[:], cW0_d[:])
            cW1_t = cst.tile([H1, H1], gdt)
            nc.sync.dma_start(cW1_t[:], cW1_d[:])
            fW0_t = cst.tile([H1, H1], f32)
            nc.sync.dma_start(fW0_t[:], fW0_d[:])
            fb0_t = cst.tile([H1, 1], f32)
            nc.sync.dma_start(fb0_t[:], fb0_d[:])
            fW1_t = cst.tile([H1, 32], f32)
            nc.sync.dma_start(fW1_t[:], fW1_d[:])
            fb1_t = cst.tile([32, 1], f32)
            nc.sync.dma_start(fb1_t[:], fb1_d[:])
            fW2_t = cst.tile([32, 1], f32)
            nc.sync.dma_start(fW2_t[:], fW2_d[:])
            fb2_t = cst.tile([1, 1], f32)
            nc.sync.dma_start(fb2_t[:], fb2_d[:])

            T1_shard = dram.tile([SLOTS, 128], gdt)
            T2_shard = dram.tile([SLOTS, 128], gdt)
            T1_full = dram.tile([NROWS, 128], gdt, addr_space="Shared")
            T2_full = dram.tile([NROWS, 128], gdt, addr_space="Shared")

            # ---------------- stage: T1 = dinv * (x @ cW0), own shard
            # batched: one input DMA and one strided output DMA per 7 blocks
            TB = 7
            for bb in range(0, BLKN, TB):
                nb = min(TB, BLKN - bb)
                xpt = hp.tile([128, TB * 128], gdt, tag="xpt")
                nc.sync.dma_start(
                    xpt[:, : nb * 128],
                    xpT_d[:, bb * 128:(bb + nb) * 128],
                )
                t1b = hp.tile([128, TB, H1], gdt, tag="t1b")
                for j in range(nb):
                    b = bb + j
                    psx = ps_run.tile([128, H1], f32, tag=f"agg{b % G}")
                    nc.tensor.matmul(
                        psx[:], xpt[:, j * 128:(j + 1) * 128], cW0_t[:],
                        start=True, stop=True,
                    )
                    nc.scalar.activation(
                        t1b[:, j, :], psx[:],
                        mybir.ActivationFunctionType.Copy,
                        scale=dinv_t[:, b:b + 1],
                    )
                dst = T1_shard[:, :]
                dst3 = bass.AP(
                    dst.tensor, dst.offset + bb * 128 * 128,
                    [[128, 128], [128 * 128, nb], [1, H1]],
                )
                nc.sync.dma_start(dst3, t1b[:, :nb, :])

            def load_town(table_shard, tag):
                # bulk-load the whole own shard once: town_all[p, b, f] =
                # shard[b*128+p, f] (strided DMA, no per-block loads)
                town_all = twp.tile(
                    [128, BLKN, H1], gdt, tag=f"townall{tag}",
                    name=f"townall{tag}",
                )
                sh = table_shard[:, :]
                src3 = bass.AP(
                    sh.tensor, sh.offset,
                    [[128, 128], [128 * 128, BLKN], [1, H1]],
                )
                nc.sync.dma_start(town_all[:], src3)
                return town_all

            town1 = load_town(T1_shard, "1")

            nc.gpsimd.collective_compute(
                "AllGather",
                mybir.AluOpType.bypass,
                ins=[T1_shard.opt()],
                outs=[T1_full.opt()],
                replica_groups=[list(range(NCORES))],
            )

            def emit_layer(table_full, town_all, phase_a, phase_b, tag):
                qn = [0]

                def agg_psum(b):
                    ps_b = ps_run.tile(
                        [128, H1], f32, tag=f"agg{b % G}", name=f"agg{tag}_{b}"
                    )
                    nc.tensor.matmul(
                        ps_b[:], identb_t[:], town_all[:, b, :],
                        start=True, stop=(last_tile[b] < 0),
                    )
                    return ps_b

                psums = {}
                deferred = []
                ngrp = len(chunks_by_group)
                for gi, (blocks, gch) in enumerate(chunks_by_group):
                    # in the last 2 groups there is no downstream aggregation
                    # to protect: run phase B inline to shorten the tail
                    eager = gi >= ngrp - 2
                    for b in blocks:
                        psums[b] = agg_psum(b)
                        if last_tile[b] < 0:
                            deferred.append((b, phase_a(b, psums.pop(b))))
                    for ci, (t0, k, be, off) in enumerate(gch):
                        if ci == 2 or eager:
                            for (b2, h2) in deferred:
                                phase_b(b2, h2)
                            deferred = []
                        comb_t = iop.tile(
                            [128, CH * 9], mybir.dt.int16, tag=f"idx{tag}"
                        )
                        nc.sync.dma_start(
                            comb_t[:, : k * 9], comb_d[:, off: off + 9 * k]
                        )
                        gat = gatp.tile([128, CH, 128], gdt, tag="gat")
                        nc.gpsimd.dma_gather(
                            gat[:, :k, :],
                            table_full[be * BSIZE: (be + 1) * BSIZE, :],
                            comb_t[:, : k * 8], k * 128, k * 128, 128,
                            queue_num=qn[0] % 4, single_packet=False,
                        )
                        qn[0] += 1
                        # one broadcast is_equal builds all k one-hots at once
                        s_big = sp.tile([128, CH * 128], gdt, tag="s_t")
                        mt = comb_t[:, k * 8: k * 9].bitcast(gdt)
                        meta_bc = bass.AP(
                            mt.tensor, mt.offset,
                            [list(mt.ap[0]), [mt.ap[1][0], k], [0, 128]],
                        )
                        sb = s_big[:, : k * 128]
                        sb3 = bass.AP(
                            sb.tensor, sb.offset,
                            [list(sb.ap[0]), [128, k], [1, 128]],
                        )
                        io = iota_t[:, : k * 128]
                        io3 = bass.AP(
                            io.tensor, io.offset,
                            [list(io.ap[0]), [128, k], [1, 128]],
                        )
                        nc.vector.tensor_tensor(
                            sb3, io3, meta_bc, mybir.AluOpType.is_equal
                        )
                        for tl in range(k):
                            ti = t0 + tl
                            b = int(tile_to_block[ti])
                            is_last = ti == last_tile[b]
                            nc.tensor.matmul(
                                psums[b][:],
                                s_big[:, tl * 128:(tl + 1) * 128],
                                gat[:, tl, :H1],
                                start=False, stop=is_last,
                            )
                            if is_last:
                                h = phase_a(b, psums.pop(b))
                                if eager:
                                    phase_b(b, h)
                                else:
                                    deferred.append((b, h))
                for (b2, h2) in deferred:
                    phase_b(b2, h2)

            # ---------------- layer 1 (phase B builds T2 rows)
            def pa1(b, ps_b):
                h1s = hp.tile([128, H1], f32, tag="h1s")
                nc.scalar.activation(
                    h1s[:], ps_b[:], mybir.ActivationFunctionType.Tanh,
                    scale=dinv_t[:, b:b + 1],
                )
                return h1s

            def pb1(b, h1s):
                tp = ps_epi.tile([H1, 128], f32, tag="eps")
                nc.tensor.transpose(tp[:], h1s[:], identf_t[:])
                h1Tb = hp.tile([H1, 128], gdt, tag="h1Tb")
                nc.vector.tensor_copy(h1Tb[:], tp[:])
                ps2 = ps_epi.tile([128, H1], f32, tag="eps")
                nc.tensor.matmul(ps2[:], h1Tb[:], cW1_t[:], start=True, stop=True)
                t2n = hp.tile([128, H1], gdt, tag="t1n")
                nc.scalar.activation(
                    t2n[:], ps2[:], mybir.ActivationFunctionType.Copy,
                    scale=dinv_t[:, b:b + 1],
                )
                nc.sync.dma_start(T2_shard[b * 128:(b + 1) * 128, :H1], t2n[:])

            emit_layer(T1_full, town1, pa1, pb1, "1")

            town2 = load_town(T2_shard, "2")

            nc.gpsimd.collective_compute(
                "AllGather",
                mybir.AluOpType.bypass,
                ins=[T2_shard.opt()],
                outs=[T2_full.opt()],
                replica_groups=[list(range(NCORES))],
            )

            # ---------------- layer 2 (phase B runs the FC head)
            def pa2(b, ps_b):
                h2s = hp.tile([128, H1], f32, tag="h1s")
                nc.scalar.activation(
                    h2s[:], ps_b[:], mybir.ActivationFunctionType.Tanh,
                    scale=dinv_t[:, b:b + 1],
                )
                return h2s

            def pb2(b, h2s):
                tp = ps_epi.tile([H1, 128], f32, tag="eps")
                nc.tensor.transpose(tp[:], h2s[:], identf_t[:])
                h2T = hp.tile([H1, 128], f32, tag="h2T")
                nc.vector.tensor_copy(h2T[:], tp[:])
                e2 = ps_epi.tile([H1, 128], f32, tag="eps")
                nc.tensor.matmul(e2[:], fW0_t[:], h2T[:], start=True, stop=True)
                h3T = hp.tile([H1, 128], f32, tag="h3T")
                nc.scalar.activation(
                    h3T[:], e2[:], mybir.ActivationFunctionType.Tanh,
                    bias=fb0_t[:, 0:1],
                )
                e3 = ps_epi.tile([32, 128], f32, tag="eps")
                nc.tensor.matmul(e3[:], fW1_t[:], h3T[:], start=True, stop=True)
                h4T = hp.tile([32, 128], f32, tag="h4T")
                nc.scalar.activation(
                    h4T[:], e3[:], mybir.ActivationFunctionType.Tanh,
                    bias=fb1_t[:, 0:1],
                )
                e4 = ps_epi.tile([1, 128], f32, tag="eps")
                nc.tensor.matmul(e4[:], fW2_t[:], h4T[:], start=True, stop=True)
                yrow = hp.tile([1, 128], f32, tag="yrow")
                nc.vector.tensor_scalar_add(yrow[:], e4[:], fb2_t[0:1, 0:1])
                nc.sync.dma_start(y_d[b:b + 1, :], yrow[:])

            emit_layer(T2_full, town2, pa2, pb2, "2")

    nc.compile()
    return nc


# ------------------------------------------------------------------- driver
def kernel(**inputs):
    global LAST_EXEC_TIME_NS, LAST_RESULTS

    x = np.ascontiguousarray(np.asarray(inputs["x"], np.float32))
    ei = np.asarray(inputs["edge_index"], np.int64)
    src, dst = ei[0], ei[1]

    cb0 = np.asarray(inputs["cb0"], np.float32)
    cb1 = np.asarray(inputs["cb1"], np.float32)
    assert not (np.any(cb0) or np.any(cb1)), "nonzero conv bias unsupported"

    plan, streams, xpTs, dinvbs = _plan(src, dst, x)

    nc = _build_program(plan)

    iota = np.tile(np.arange(128, dtype=np.float32), (128, CH))
    iota = np.ascontiguousarray(iota.astype(ml_dtypes.bfloat16))
    identb = np.eye(128, dtype=np.float32).astype(ml_dtypes.bfloat16)
    identf = np.eye(128, dtype=np.float32)

    common = {
        "iota": iota, "identb": identb, "identf": identf,
        "cW0b": np.ascontiguousarray(
            np.asarray(inputs["cW0"], np.float32).astype(ml_dtypes.bfloat16)
        ),
        "cW1b": np.ascontiguousarray(
            np.asarray(inputs["cW1"], np.float32).astype(ml_dtypes.bfloat16)
        ),
        "fW0": np.ascontiguousarray(np.asarray(inputs["fW0"], np.float32)),
        "fb0": np.asarray(inputs["fb0"], np.float32).reshape(H1, 1),
        "fW1": np.ascontiguousarray(np.asarray(inputs["fW1"], np.float32)),
        "fb1": np.asarray(inputs["fb1"], np.float32).reshape(32, 1),
        "fW2": np.ascontiguousarray(np.asarray(inputs["fW2"], np.float32)),
        "fb2": np.asarray(inputs["fb2"], np.float32).reshape(1, 1),
    }

    in_maps = []
    for c in range(NCORES):
        m = {"xpT": xpTs[c], "comb": streams[c], "dinvb": dinvbs[c]}
        m.update(common)
        in_maps.append(m)

    trace = os.environ.get("BASS_GCN_TRACE") == "1"
    res = run_bass_kernel_spmd(nc, in_maps, list(range(NCORES)), trace=trace)
    if trace:
        LAST_EXEC_TIME_NS = res.exec_time_ns
    LAST_RESULTS = res

    node_of_pos = plan["node_of_pos"]
    out = np.zeros((N, 1), np.float32)
    for c in range(NCORES):
        yflat = res.results[c]["y"].reshape(SLOTS)
        valid = node_of_pos[c] >= 0
        out[node_of_pos[c][valid], 0] = yflat[valid]
    return out


# revision 52
# speedup vs baseline: 1.0509x; 1.0509x over previous
"""BrainGCN Trainium2 kernel v2: 2x GCNConv + 3 FC layers over a 100K-node,
1.6M-edge random graph, distributed over 8 NeuronCores.

Strategy (v2, transform-first + binary one-hot):
- Nodes (dst) sharded across 8 cores; per core 12544 slots in 98 blocks of 128,
  assigned by capped vector-LPT (per-bucket in-edge counts balanced, hard cap
  512 per (bucket, block) so most groups need exactly 4 tiles).
- Transform-first: each layer gathers from a table T_l[v] = dinv[v]*(h@W)[v]
  (row-major, 64 real bf16 feats in 256B rows; upper half garbage, never read).
  Tables are device-built (PE matmul per block + scalar-engine scale) and
  exchanged with AllGather. dinv[src] is folded into the table, dinv[dst] is a
  per-partition activation scale, the self-loop is "one more edge" (identity
  matmul over the core's own shard rows), so the per-edge one-hot S is BINARY:
  one cheap DVE is_equal per tile, no per-edge weights anywhere.
- Aggregation runs slot-major: psum[slot, feat] += S_t.T @ G_t for all tiles of
  a block in ONE psum run (no SBUF accumulators). Blocks processed in groups of
  6 so gather calls chunk long same-bucket segments.
- Buckets are 4x25088 rows (= 2 shards) so an edge's bucket is src_core//2 —
  known before LPT, and int16 gather indices stay in range.
- Messages fetched with gpsimd dma_gather (256B/edge), CH tiles per call,
  single_packet=False, round-robined over 4 SWDGE queues.

Host-side work is limited to graph-structure preprocessing (degrees, norms,
permutations, index/metadata arrays, row reordering/casting of x) and final
unpermutation.
"""

import os
import sys
import types

import numpy as np


def _install_ntff_hook():
    """Image's antenv lacks axon_hooks; shim it so trace=True can profile."""
    if "antenv.axon_hooks" in sys.modules:
        return
    mod = types.ModuleType("antenv.axon_hooks")
    mod._hook = None
    mod.set_axon_ntff_profile_hook = lambda h: setattr(mod, "_hook", h)
    mod.get_axon_ntff_profile_hook = lambda: mod._hook
    sys.modules["antenv.axon_hooks"] = mod
    try:
        import antenv
        antenv.axon_hooks = mod
        from trn_agent_boot.trn_boot import _ntff_profile_via_ctypes
        mod.set_axon_ntff_profile_hook(
            _ntff_profile_via_ctypes("/opt/axon/libaxon_pjrt.so")
        )
    except Exception:
        pass


_install_ntff_hook()

import ml_dtypes
import concourse.bacc as bacc
import concourse.bass as bass  # noqa: F401
import concourse.mybir as mybir
import concourse.tile as tile
from concourse.bass_utils import run_bass_kernel_spmd

# ---------------------------------------------------------------- constants
N = 100000
D_IN = 128
H1 = 64
NCORES = 8
SHARD = N // NCORES            # 12500
BLKN = 98                      # blocks of 128 slots per core
SLOTS = BLKN * 128             # 12544
NROWS = NCORES * SLOTS         # 100352 table rows
NBUCK = 4
BSIZE = 2 * SLOTS              # 25088 rows per bucket (= 2 src shards)
CAP = 512                      # LPT hard cap per (bucket, block): 4 tiles
G = int(os.environ.get("BASS_GCN_G", "6"))      # psum rotation depth
SG = int(os.environ.get("BASS_GCN_SG", "3"))     # blocks per schedule group
CH = int(os.environ.get("BASS_GCN_CH", "16"))   # tiles per dma_gather call
GATB = int(os.environ.get("BASS_GCN_GATB", "18"))  # gather tile bufs

LAST_EXEC_TIME_NS = None       # filled when BASS_GCN_TRACE=1
LAST_RESULTS = None


# ------------------------------------------------------------- host planning
def _lpt_assign_vec(loads, capsT):
    """Pack nodes into BLKN blocks x 128 slots, balancing the per-bucket load
    vectors (sum-of-squares greedy, descending total load) under per-(bucket,
    block) caps (a few designated overflow blocks get a 640 cap)."""
    n = loads.shape[0]
    order = np.argsort(-loads.sum(1), kind="stable")
    block_loads = np.zeros((BLKN, loads.shape[1]), np.float64)
    used = np.zeros(BLKN, np.int64)
    pos = np.empty(n, np.int64)
    for i in order:
        li = loads[i]
        cand = block_loads + li
        score = np.einsum("ij,ij->i", cand, cand)
        over = (cand > capsT).sum(axis=1)
        score = score + over * 1e12
        score[used >= 128] = np.inf
        b = int(np.argmin(score))
        pos[i] = b * 128 + used[b]
        block_loads[b] += li
        used[b] += 1

    # swap-repair: drive every (block, bucket) under its cap
    blk_of = pos // 128
    stuck = np.zeros((BLKN, loads.shape[1]), bool)
    for _ in range(8000):
        over = np.where(stuck, -1.0, block_loads - capsT)
        b = int(np.argmax(over.max(1)))
        be = int(np.argmax(over[b]))
        if over[b, be] <= 0:
            break
        in_b = np.nonzero(blk_of == b)[0]
        cand_i = in_b[np.argsort(-loads[in_b, be])[:8]]
        done = False
        for b2 in np.argsort(block_loads[:, be]):
            if b2 == b:
                continue
            in_b2 = np.nonzero(blk_of == b2)[0]
            cand_j = in_b2[np.argsort(loads[in_b2, be])[:8]]
            for i in cand_i:
                for j in cand_j:
                    dlt = loads[i] - loads[j]
                    if dlt[be] <= 0:
                        continue
                    nb2 = block_loads[b2] + dlt
                    nb = block_loads[b] - dlt
                    if (nb2 <= capsT[b2]).all() and nb[be] < block_loads[b, be]:
                        block_loads[b2] = nb2
                        block_loads[b] = nb
                        blk_of[i], blk_of[j] = b2, b
                        done = True
                        break
                if done:
                    break
            if done:
                break
        if not done:
            stuck[b, be] = True
    # reassign slots from (possibly updated) block membership
    used = np.zeros(BLKN, np.int64)
    for i in range(n):
        b = blk_of[i]
        pos[i] = b * 128 + used[b]
        used[b] += 1
    return pos


def _plan(src, dst, x):
    """Host-side graph preprocessing. Returns per-core stream data plus the
    shared tile schedule."""
    deg = (np.bincount(dst, minlength=N) + 1.0).astype(np.float64)
    dinv = (1.0 / np.sqrt(deg)).astype(np.float32)

    src_core = src // SHARD
    be_e = src_core // 2                       # bucket of every edge (0..3)

    loads = np.zeros((N, NBUCK), np.float64)
    np.add.at(loads, (dst, be_e), 1.0)

    # overflow blocks: enough 640-cap blocks per bucket to absorb every
    # core's excess over 98x512, +1 margin; disjoint across buckets
    totals = np.zeros((NCORES, NBUCK))
    for c in range(NCORES):
        totals[c] = loads[c * SHARD:(c + 1) * SHARD].sum(0)
    kov = np.maximum(
        0, np.ceil((totals.max(0) - BLKN * CAP) / 128)
    ).astype(int) + 1
    caps = np.full((NBUCK, BLKN), float(CAP))
    m0 = 0
    for be in range(NBUCK):
        caps[be, m0:m0 + kov[be]] = CAP + 128.0
        m0 += kov[be]
    capsT = np.ascontiguousarray(caps.T)

    pos_local = np.empty(N, np.int64)
    node_of_pos = np.full((NCORES, SLOTS), -1, np.int64)
    for c in range(NCORES):
        nodes = np.arange(c * SHARD, (c + 1) * SHARD)
        p = _lpt_assign_vec(loads[nodes], capsT)
        pos_local[nodes] = p
        node_of_pos[c, p] = nodes
    pos_global = (np.arange(N) // SHARD) * SLOTS + pos_local

    rows = pos_global[src]                     # table row per edge
    assert (rows // BSIZE == be_e).all()
    idx_local = rows - be_e * BSIZE            # 0..25087, int16-safe

    core_of = dst // SHARD
    counts = np.zeros((NCORES, NBUCK, BLKN), np.int64)
    for c in range(NCORES):
        m = core_of == c
        blk = pos_local[dst[m]] // 128
        counts[c] = np.bincount(
            be_e[m] * BLKN + blk, minlength=NBUCK * BLKN
        ).reshape(NBUCK, BLKN)

    T = np.ceil(counts.max(axis=0) / 128).astype(np.int64)   # [NBUCK, BLKN]

    # ---- schedule: groups of G blocks, bucket-sweep inside each group
    sched = []                                  # (be, b) emission order
    for g0 in range(0, BLKN, SG):
        blocks = range(g0, min(g0 + SG, BLKN))
        for be in range(NBUCK):
            for b in blocks:
                sched.append((be, b))
    tile_off = np.zeros((NBUCK, BLKN), np.int64)
    off = 0
    for (be, b) in sched:
        tile_off[be, b] = off
        off += T[be, b]
    ntiles = int(off)
    P = 128 * ntiles

    tile_to_block = np.zeros(ntiles, np.int64)
    for (be, b) in sched:
        tile_to_block[tile_off[be, b]: tile_off[be, b] + T[be, b]] = b
    last_tile_of_block = np.full(BLKN, -1, np.int64)
    for (be, b) in sched:
        if T[be, b] > 0:
            last_tile_of_block[b] = tile_off[be, b] + T[be, b] - 1

    # ---- gather chunk lists per group: (tile_start, k, be, comb_off)
    chunks_by_group = []
    comb_off = 0
    for g0 in range(0, BLKN, SG):
        blocks = list(range(g0, min(g0 + SG, BLKN)))
        gch = []
        for be in range(NBUCK):
            seg_len = int(sum(T[be, b] for b in blocks))
            if seg_len == 0:
                continue
            seg_start = int(tile_off[be, blocks[0]])
            t = 0
            while t < seg_len:
                k = min(CH, seg_len - t)
                gch.append((seg_start + t, k, be, comb_off))
                comb_off += 9 * k
                t += k
        chunks_by_group.append((blocks, gch))
    comb_len = comb_off

    # ---- per-core streams
    group_idx = np.zeros((NBUCK, BLKN), np.int64)
    for i, (be, b) in enumerate(sched):
        group_idx[be, b] = i
    dest_base = np.zeros(len(sched) + 1, np.int64)
    np.cumsum([128 * T[be, b] for (be, b) in sched], out=dest_base[1:])

    streams = []
    xpTs = []
    dinvbs = []
    for c in range(NCORES):
        m = core_of == c
        e_rows = idx_local[m]
        e_be = be_e[m]
        e_slot = pos_local[dst[m]]
        key = group_idx[e_be, e_slot // 128]
        order = np.lexsort((e_rows, key))

        skey = key[order]
        cnt = np.bincount(skey, minlength=len(sched))
        starts = np.zeros(len(sched) + 1, np.int64)
        np.cumsum(cnt, out=starts[1:])
        rank = np.arange(len(order)) - starts[skey]
        dest = dest_base[skey] + rank

        out_idx = np.zeros(P, np.int64)
        out_slot = np.full(P, -1.0, np.float32)
        out_idx[dest] = e_rows[order]
        out_slot[dest] = (e_slot[order] % 128).astype(np.float32)

        idx_wrapped = np.tile(
            out_idx.astype(np.int16).reshape(-1, 16).T, (8, 1)
        )                                           # [128, P//16]
        meta = out_slot.reshape(ntiles, 128).T.astype(
            ml_dtypes.bfloat16
        ).view(np.int16)                            # [128, ntiles]
        # combined per-chunk stream: [idx (k*8 cols) | meta (k cols)] int16
        comb = np.zeros((128, comb_len), np.int16)
        for (t0, k, be, off) in [c for (_, gch) in chunks_by_group
                                 for c in gch]:
            comb[:, off: off + 8 * k] = idx_wrapped[:, t0 * 8: (t0 + k) * 8]
            comb[:, off + 8 * k: off + 9 * k] = meta[:, t0: t0 + k]
        streams.append(np.ascontiguousarray(comb))

        xp = np.zeros((SLOTS, D_IN), np.float32)
        valid = node_of_pos[c] >= 0
        xp[valid] = x[node_of_pos[c][valid]]
        xpTs.append(np.ascontiguousarray(xp.T.astype(ml_dtypes.bfloat16)))

        dv = np.zeros(SLOTS, np.float32)
        dv[valid] = dinv[node_of_pos[c][valid]]
        dinvbs.append(np.ascontiguousarray(dv.reshape(BLKN, 128).T))

    plan = {
        "T": T, "ntiles": ntiles, "P": P, "comb_len": comb_len,
        "chunks_by_group": chunks_by_group,
        "tile_to_block": tile_to_block,
        "last_tile_of_block": last_tile_of_block,
        "node_of_pos": node_of_pos,
    }
    return plan, streams, xpTs, dinvbs


# ------------------------------------------------------------ device program
def _build_program(plan):
    f32 = mybir.dt.float32
    gdt = mybir.dt.bfloat16
    scratch = int(os.environ.get("BASS_GCN_SCRATCH", "16384"))
    nc = bacc.Bacc(
        "TRN2", num_swdge_queues=4, dynamic_dma_scratch_size=scratch
    )

    ntiles = plan["ntiles"]
    P = plan["P"]
    chunks_by_group = plan["chunks_by_group"]
    tile_to_block = plan["tile_to_block"]
    last_tile = plan["last_tile_of_block"]

    comb_len = plan["comb_len"]
    xpT_d = nc.dram_tensor("xpT", [D_IN, SLOTS], gdt, kind="ExternalInput")
    comb_d = nc.dram_tensor(
        "comb", [128, comb_len], mybir.dt.int16, kind="ExternalInput"
    )
    dinv_d = nc.dram_tensor("dinvb", [128, BLKN], f32, kind="ExternalInput")
    iota_d = nc.dram_tensor("iota", [128, CH * 128], gdt, kind="ExternalInput")
    identb_d = nc.dram_tensor("identb", [128, 128], gdt, kind="ExternalInput")
    identf_d = nc.dram_tensor("identf", [128, 128], f32, kind="ExternalInput")
    cW0_d = nc.dram_tensor("cW0b", [D_IN, H1], gdt, kind="ExternalInput")
    cW1_d = nc.dram_tensor("cW1b", [H1, H1], gdt, kind="ExternalInput")
    fW0_d = nc.dram_tensor("fW0", [H1, H1], f32, kind="ExternalInput")
    fb0_d = nc.dram_tensor("fb0", [H1, 1], f32, kind="ExternalInput")
    fW1_d = nc.dram_tensor("fW1", [H1, 32], f32, kind="ExternalInput")
    fb1_d = nc.dram_tensor("fb1", [32, 1], f32, kind="ExternalInput")
    fW2_d = nc.dram_tensor("fW2", [32, 1], f32, kind="ExternalInput")
    fb2_d = nc.dram_tensor("fb2", [1, 1], f32, kind="ExternalInput")
    y_d = nc.dram_tensor("y", [BLKN, 128], f32, kind="ExternalOutput")

    with tile.TileContext(nc) as tc:
        with (
            tc.tile_pool(name="cst", bufs=1) as cst,
            tc.tile_pool(name="gatp", bufs=GATB) as gatp,
            tc.tile_pool(name="sp", bufs=10) as sp,
            tc.tile_pool(name="iop", bufs=24) as iop,
            tc.tile_pool(name="hp", bufs=8) as hp,
            tc.tile_pool(name="twp", bufs=1) as twp,
            tc.tile_pool(name="ps_run", bufs=1, space="PSUM") as ps_run,
            tc.tile_pool(name="ps_epi", bufs=2, space="PSUM") as ps_epi,
            tc.tile_pool(name="dram", bufs=1, space="DRAM") as dram,
        ):
            iota_t = cst.tile([128, CH * 128], gdt)
            nc.sync.dma_start(iota_t[:], iota_d[:])
            identb_t = cst.tile([128, 128], gdt)
            nc.sync.dma_start(identb_t[:], identb_d[:])
            identf_t = cst.tile([128, 128], f32)
            nc.sync.dma_start(identf_t[:], identf_d[:])
            dinv_t = cst.tile([128, BLKN], f32)
            nc.sync.dma_start(dinv_t[:], dinv_d[:])
            cW0_t = cst.tile([D_IN, H1], gdt)
            nc.sync.dma_start(cW0_t[:], cW0_d[:])
            cW1_t = cst.tile([H1, H1], gdt)
            nc.sync.dma_start(cW1_t[:], cW1_d[:])
            fW0_t = cst.tile([H1, H1], f32)
            nc.sync.dma_start(fW0_t[:], fW0_d[:])
            fb0_t = cst.tile([H1, 1], f32)
            nc.sync.dma_start(fb0_t[:], fb0_d[:])
            fW1_t = cst.tile([H1, 32], f32)
            nc.sync.dma_start(fW1_t[:], fW1_d[:])
            fb1_t = cst.tile([32, 1], f32)
            nc.sync.dma_start(fb1_t[:], fb1_d[:])
            fW2_t = cst.tile([32, 1], f32)
            nc.sync.dma_start(fW2_t[:], fW2_d[:])
            fb2_t = cst.tile([1, 1], f32)
            nc.sync.dma_start(fb2_t[:], fb2_d[:])

            T1_shard = dram.tile([SLOTS, 128], gdt)
            T2_shard = dram.tile([SLOTS, 128], gdt)
            T1_full = dram.tile([NROWS, 128], gdt, addr_space="Shared")
            T2_full = dram.tile([NROWS, 128], gdt, addr_space="Shared")

            # ---------------- stage: T1 = dinv * (x @ cW0), own shard
            # batched: one input DMA and one strided output DMA per 7 blocks
            TB = 7
            for bb in range(0, BLKN, TB):
                nb = min(TB, BLKN - bb)
                xpt = hp.tile([128, TB * 128], gdt, tag="xpt")
                nc.sync.dma_start(
                    xpt[:, : nb * 128],
                    xpT_d[:, bb * 128:(bb + nb) * 128],
                )
                t1b = hp.tile([128, TB, H1], gdt, tag="t1b")
                for j in range(nb):
                    b = bb + j
                    psx = ps_run.tile([128, H1], f32, tag=f"agg{b % G}")
                    nc.tensor.matmul(
                        psx[:], xpt[:, j * 128:(j + 1) * 128], cW0_t[:],
                        start=True, stop=True,
                    )
                    nc.scalar.activation(
                        t1b[:, j, :], psx[:],
                        mybir.ActivationFunctionType.Copy,
                        scale=dinv_t[:, b:b + 1],
                    )
                dst = T1_shard[:, :]
                dst3 = bass.AP(
                    dst.tensor, dst.offset + bb * 128 * 128,
                    [[128, 128], [128 * 128, nb], [1, H1]],
                )
                nc.sync.dma_start(dst3, t1b[:, :nb, :])

            def load_town(table_shard, tag):
                # bulk-load the whole own shard once: town_all[p, b, f] =
                # shard[b*128+p, f] (strided DMA, no per-block loads)
                town_all = twp.tile(
                    [128, BLKN, H1], gdt, tag=f"townall{tag}",
                    name=f"townall{tag}",
                )
                sh = table_shard[:, :]
                src3 = bass.AP(
                    sh.tensor, sh.offset,
                    [[128, 128], [128 * 128, BLKN], [1, H1]],
                )
                nc.sync.dma_start(town_all[:], src3)
                return town_all

            town1 = load_town(T1_shard, "1")

            nc.gpsimd.collective_compute(
                "AllGather",
                mybir.AluOpType.bypass,
                ins=[T1_shard.opt()],
                outs=[T1_full.opt()],
                replica_groups=[list(range(NCORES))],
            )

            def emit_layer(table_full, town_all, phase_a, phase_b, tag):
                qn = [0]

                def agg_psum(b):
                    ps_b = ps_run.tile(
                        [128, H1], f32, tag=f"agg{b % G}", name=f"agg{tag}_{b}"
                    )
                    nc.tensor.matmul(
                        ps_b[:], identb_t[:], town_all[:, b, :],
                        start=True, stop=(last_tile[b] < 0),
                    )
                    return ps_b

                psums = {}
                deferred = []
                ngrp = len(chunks_by_group)
                for gi, (blocks, gch) in enumerate(chunks_by_group):
                    # in the last 2 groups there is no downstream aggregation
                    # to protect: run phase B inline to shorten the tail
                    eager = gi >= ngrp - 2
                    for b in blocks:
                        psums[b] = agg_psum(b)
                        if last_tile[b] < 0:
                            deferred.append((b, phase_a(b, psums.pop(b))))
                    for ci, (t0, k, be, off) in enumerate(gch):
                        if eager:
                            for (b2, h2) in deferred:
                                phase_b(b2, h2)
                            deferred = []
                        elif ci >= 1 and deferred:
                            phase_b(*deferred.pop(0))
                        comb_t = iop.tile(
                            [128, CH * 9], mybir.dt.int16, tag=f"idx{tag}"
                        )
                        nc.sync.dma_start(
                            comb_t[:, : k * 9], comb_d[:, off: off + 9 * k]
                        )
                        gat = gatp.tile([128, CH, 128], gdt, tag="gat")
                        nc.gpsimd.dma_gather(
                            gat[:, :k, :],
                            table_full[be * BSIZE: (be + 1) * BSIZE, :],
                            comb_t[:, : k * 8], k * 128, k * 128, 128,
                            queue_num=qn[0] % 4, single_packet=False,
                        )
                        qn[0] += 1
                        # one broadcast is_equal builds all k one-hots at once
                        s_big = sp.tile([128, CH * 128], gdt, tag="s_t")
                        mt = comb_t[:, k * 8: k * 9].bitcast(gdt)
                        meta_bc = bass.AP(
                            mt.tensor, mt.offset,
                            [list(mt.ap[0]), [mt.ap[1][0], k], [0, 128]],
                        )
                        sb = s_big[:, : k * 128]
                        sb3 = bass.AP(
                            sb.tensor, sb.offset,
                            [list(sb.ap[0]), [128, k], [1, 128]],
                        )
                        io = iota_t[:, : k * 128]
                        io3 = bass.AP(
                            io.tensor, io.offset,
                            [list(io.ap[0]), [128, k], [1, 128]],
                        )
                        nc.vector.tensor_tensor(
                            sb3, io3, meta_bc, mybir.AluOpType.is_equal
                        )
                        for tl in range(k):
                            ti = t0 + tl
                            b = int(tile_to_block[ti])
                            is_last = ti == last_tile[b]
                            nc.tensor.matmul(
                                psums[b][:],
                                s_big[:, tl * 128:(tl + 1) * 128],
                                gat[:, tl, :H1],
                                start=False, stop=is_last,
                            )
                            if is_last:
                                h = phase_a(b, psums.pop(b))
                                if eager:
                                    phase_b(b, h)
                                else:
                                    deferred.append((b, h))
                for (b2, h2) in deferred:
                    phase_b(b2, h2)

            # ---------------- layer 1 (phase B builds T2 rows)
            def pa1(b, ps_b):
                h1s = hp.tile([128, H1], f32, tag="h1s")
                nc.scalar.activation(
                    h1s[:], ps_b[:], mybir.ActivationFunctionType.Tanh,
                    scale=dinv_t[:, b:b + 1],
                )
                return h1s

            def pb1(b, h1s):
                tp = ps_epi.tile([H1, 128], f32, tag="eps")
                nc.tensor.transpose(tp[:], h1s[:], identf_t[:])
                h1Tb = hp.tile([H1, 128], gdt, tag="h1Tb")
                nc.vector.tensor_copy(h1Tb[:], tp[:])
                ps2 = ps_epi.tile([128, H1], f32, tag="eps")
                nc.tensor.matmul(ps2[:], h1Tb[:], cW1_t[:], start=True, stop=True)
                t2n = hp.tile([128, H1], gdt, tag="t1n")
                nc.scalar.activation(
                    t2n[:], ps2[:], mybir.ActivationFunctionType.Copy,
                    scale=dinv_t[:, b:b + 1],
                )
                nc.sync.dma_start(T2_shard[b * 128:(b + 1) * 128, :H1], t2n[:])

            emit_layer(T1_full, town1, pa1, pb1, "1")

            town2 = load_town(T2_shard, "2")

            nc.gpsimd.collective_compute(
                "AllGather",
                mybir.AluOpType.bypass,
                ins=[T2_shard.opt()],
                outs=[T2_full.opt()],
                replica_groups=[list(range(NCORES))],
            )

            # ---------------- layer 2 (phase B runs the FC head)
            def pa2(b, ps_b):
                h2s = hp.tile([128, H1], f32, tag="h1s")
                nc.scalar.activation(
                    h2s[:], ps_b[:], mybir.ActivationFunctionType.Tanh,
                    scale=dinv_t[:, b:b + 1],
                )
                return h2s

            def pb2(b, h2s):
                tp = ps_epi.tile([H1, 128], f32, tag="eps")
                nc.tensor.transpose(tp[:], h2s[:], identf_t[:])
                h2T = hp.tile([H1, 128], f32, tag="h2T")
                nc.vector.tensor_copy(h2T[:], tp[:])
                e2 = ps_epi.tile([H1, 128], f32, tag="eps")
                nc.tensor.matmul(e2[:], fW0_t[:], h2T[:], start=True, stop=True)
                h3T = hp.tile([H1, 128], f32, tag="h3T")
                nc.scalar.activation(
                    h3T[:], e2[:], mybir.ActivationFunctionType.Tanh,
                    bias=fb0_t[:, 0:1],
                )
                e3 = ps_epi.tile([32, 128], f32, tag="eps")
                nc.tensor.matmul(e3[:], fW1_t[:], h3T[:], start=True, stop=True)
                h4T = hp.tile([32, 128], f32, tag="h4T")
                nc.scalar.activation(
                    h4T[:], e3[:], mybir.ActivationFunctionType.Tanh,
                    bias=fb1_t[:, 0:1],
                )
                e4 = ps_epi.tile([1, 128], f32, tag="eps")
                nc.tensor.matmul(e4[:], fW2_t[:], h4T[:], start=True, stop=True)
                yrow = hp.tile([1, 128], f32, tag="yrow")
                nc.vector.tensor_scalar_add(yrow[:], e4[:], fb2_t[0:1, 0:1])
                nc.sync.dma_start(y_d[b:b + 1, :], yrow[:])

            emit_layer(T2_full, town2, pa2, pb2, "2")

    nc.compile()
    return nc


# ------------------------------------------------------------------- driver
def kernel(**inputs):
    global LAST_EXEC_TIME_NS, LAST_RESULTS

    x = np.ascontiguousarray(np.asarray(inputs["x"], np.float32))
    ei = np.asarray(inputs["edge_index"], np.int64)
    src, dst = ei[0], ei[1]

    cb0 = np.asarray(inputs["cb0"], np.float32)
    cb1 = np.asarray(inputs["cb1"], np.float32)
    assert not (np.any(cb0) or np.any(cb1)), "nonzero conv bias unsupported"

    plan, streams, xpTs, dinvbs = _plan(src, dst, x)

    nc = _build_program(plan)

    iota = np.tile(np.arange(128, dtype=np.float32), (128, CH))
    iota = np.ascontiguousarray(iota.astype(ml_dtypes.bfloat16))
    identb = np.eye(128, dtype=np.float32).astype(ml_dtypes.bfloat16)
    identf = np.eye(128, dtype=np.float32)

    common = {
        "iota": iota, "identb": identb, "identf": identf,
        "cW0b": np.ascontiguousarray(
            np.asarray(inputs["cW0"], np.float32).astype(ml_dtypes.bfloat16)
        ),
        "cW1b": np.ascontiguousarray(
            np.asarray(inputs["cW1"], np.float32).astype(ml_dtypes.bfloat16)
        ),
        "fW0": np.ascontiguousarray(np.asarray(inputs["fW0"], np.float32)),
        "fb0": np.asarray(inputs["fb0"], np.float32).reshape(H1, 1),
        "fW1": np.ascontiguousarray(np.asarray(inputs["fW1"], np.float32)),
        "fb1": np.asarray(inputs["fb1"], np.float32).reshape(32, 1),
        "fW2": np.ascontiguousarray(np.asarray(inputs["fW2"], np.float32)),
        "fb2": np.asarray(inputs["fb2"], np.float32).reshape(1, 1),
    }

    in_maps = []
    for c in range(NCORES):
        m = {"xpT": xpTs[c], "comb": streams[c], "dinvb": dinvbs[c]}
        m.update(common)
        in_maps.append(m)

    trace = os.environ.get("BASS_GCN_TRACE") == "1"
    res = run_bass_kernel_spmd(nc, in_maps, list(range(NCORES)), trace=trace)
    if trace:
        LAST_EXEC_TIME_NS = res.exec_time_ns
    LAST_RESULTS = res

    node_of_pos = plan["node_of_pos"]
    out = np.zeros((N, 1), np.float32)
    for c in range(NCORES):
        yflat = res.results[c]["y"].reshape(SLOTS)
        valid = node_of_pos[c] >= 0
        out[node_of_pos[c][valid], 0] = yflat[valid]
    return out
